# revision 18
# baseline (speedup 1.0000x reference)
"""CELPNet Trainium2 Bass kernel.

Strategy: pure data parallel over batch (64 -> 8 per core x 8 cores).
Per core, one Bass program computes:
  Stage A: conditioning network (pembed gather via one-hot matmul, fd1,
           conv1, conv2, fd2) in transposed layout (dims on partitions,
           batch*time on free axis), bf16 matmuls with fp32 psum.
  Stage B: phase embedding (reciprocal, prefix-scan cumsum, mod-2pi
           reduction, Sin/Cos on the ACT engine).
  Stage C: precompute A[s] = cond/phase part of the sd1 layer for all 400
           subframe steps (removes everything but the 40-dim `prev`
           feedback from the recurrent critical path).
  Stage D: 400-step recurrence (sd1-prev + sd2 + 3 GRUs + out), fully
           unrolled.  Activations live transposed: (dim-chunks on 128
           partitions, batch=8 on free).  Weights stay resident in SBUF
           as bf16 (K,M) tiles; matmuls are weight-stationary so LDWEIGHTS
           (with bf16 fast-weight-load) dominates.
All fp32 state (GRU h, outputs) is kept in fp32 master copies; bf16 only
feeds the PE.
"""

import numpy as np
import ml_dtypes

SUB, NSUB, FRAME, COND, FEAT, PEMB = 40, 4, 160, 256, 20, 64
B, T, NBF = 64, 104, 100
NCORES = 8
BC = B // NCORES          # batch per core = 8
BT = BC * T               # 832
STEPS = NBF * NSUB        # 400
TWO_PI = float(2.0 * np.pi)
BF16 = ml_dtypes.bfloat16

_cache = {}


def _build(nsteps, _dbg=False):
    import concourse.bacc as bacc
    import concourse.mybir as mybir
    import concourse.tile as tile

    dt = mybir.dt
    AF = mybir.ActivationFunctionType
    OP = mybir.AluOpType

    nc = bacc.Bacc("TRN2", target_bir_lowering=False, num_devices=NCORES)

    # ---------------- DRAM I/O ----------------
    def din(name, shape, dty=dt.bfloat16):
        return nc.dram_tensor(name, shape, dty, kind="ExternalInput")

    d_featT = din("featT", [FEAT, BT])                   # (20, 832) b-major free
    d_prow = din("prow", [1, BT], dt.float32)            # period as f32 row
    d_psc = din("psc", [BC, T], dt.float32)              # period, batch on partitions
    d_rsh = din("rsh", [BC, 1], dt.float32)
    d_pembed = din("pembed", [256, PEMB])
    d_fd1w = din("fd1w", [FEAT + PEMB, COND])
    d_c1w = din("c1w", [COND, 3 * COND])                 # [i, d*256+o]
    d_c2w = din("c2w", [COND, 3 * COND])
    d_fd2w = din("fd2w", [COND, COND])
    d_sd1c = din("sd1c", [COND, COND])                   # sd1_w rows 0:256
    d_sd1p = din("sd1p", [SUB, COND])                    # rows 256:296 (prev)
    d_sd1pr = din("sd1pr", [SUB, COND])                  # rows 296:336 (pr)
    d_sd1pi = din("sd1pi", [SUB, COND])                  # rows 336:376 (pi)
    d_sd2 = din("sd2", [COND, COND])
    d_gw = din("gw", [COND, 6 * 3 * COND])               # [i, (g*2+s)*768+o]
    d_outw = din("outw", [COND, SUB])
    # biases (fp32), packed as per-partition columns:
    # cols: fd1(2) c1(2) c2(2) fd2(2) sd1(2) sd2(2)
    d_bcols = din("bcols", [128, 12], dt.float32)
    d_gib = din("gib", [128, 18], dt.float32)            # 3 GRUs x 6 cols (bi)
    d_ghb = din("ghb", [128, 18], dt.float32)            # (bh)
    d_outb = din("outb", [SUB, 1], dt.float32)

    d_sig = nc.dram_tensor("sig", [SUB, BC * nsteps], dt.float32, kind="ExternalOutput")
    if _dbg:
        d_dbg = {}
        for nm, shp, dty in [("dbg_peT", [PEMB, BT], dt.bfloat16),
                             ("dbg_condT0", [128, BC * NBF], dt.bfloat16),
                             ("dbg_prT", [SUB, BC * STEPS], dt.bfloat16),
                             ("dbg_piT", [SUB, BC * STEPS], dt.bfloat16),
                             ("dbg_base8", [BC, STEPS], dt.float32),
                             ("dbg_w0rep", [BC, STEPS], dt.float32),
                             ("dbg_AT", [128, 2 * BC * nsteps], dt.float32),
                             ("dbg_onehot0", [128, BT], dt.bfloat16),
                             ("dbg_redc", [SUB, BC * STEPS], dt.float32),
                             ("dbg_rrc", [SUB, BC * STEPS], dt.float32),
                             ("dbg_reds", [SUB, BC * STEPS], dt.float32),
                             ("dbg_rrs", [SUB, BC * STEPS], dt.float32)]:
            d_dbg[nm] = nc.dram_tensor(nm, shp, dty, kind="ExternalOutput")
    d_h = [nc.dram_tensor(f"h{k}", [128, 16], dt.float32, kind="ExternalOutput")
           for k in range(3)]

    with tile.TileContext(nc) as tc:
        with tc.tile_pool(name="wp", bufs=1) as wp, \
             tc.tile_pool(name="work", bufs=1) as wk, \
             tc.tile_pool(name="ps", bufs=2, space="PSUM") as psp:

            def wtile(tag, shape, dty=dt.bfloat16):
                return wp.tile(shape, dty, tag=tag, name=tag)

            # ---------------- load weights to SBUF ----------------
            featT = wtile("featT", [FEAT, BT])
            nc.sync.dma_start(out=featT, in_=d_featT.ap())
            psc = wtile("psc", [BC, T], dt.float32)
            nc.sync.dma_start(out=psc, in_=d_psc.ap())
            rsh = wtile("rsh", [BC, 1], dt.float32)
            nc.sync.dma_start(out=rsh, in_=d_rsh.ap())
            pemb = [wtile(f"pemb{k}", [128, PEMB]) for k in range(2)]
            for k in range(2):
                nc.sync.dma_start(out=pemb[k], in_=d_pembed.ap()[128 * k:128 * (k + 1), :])
            fd1wA = wtile("fd1wA", [FEAT, COND])
            nc.sync.dma_start(out=fd1wA, in_=d_fd1w.ap()[0:FEAT, :])
            fd1wB = wtile("fd1wB", [PEMB, COND])
            nc.sync.dma_start(out=fd1wB, in_=d_fd1w.ap()[FEAT:FEAT + PEMB, :])
            c1w = [wtile(f"c1w{k}", [128, 3 * COND]) for k in range(2)]
            c2w = [wtile(f"c2w{k}", [128, 3 * COND]) for k in range(2)]
            fd2w = [wtile(f"fd2w{k}", [128, COND]) for k in range(2)]
            sd1c = [wtile(f"sd1c{k}", [128, COND]) for k in range(2)]
            sd2w = [wtile(f"sd2w{k}", [128, COND]) for k in range(2)]
            gw = [wtile(f"gw{k}", [128, 6 * 3 * COND]) for k in range(2)]
            outw = [wtile(f"outw{k}", [128, SUB]) for k in range(2)]
            for k in range(2):
                sl = slice(128 * k, 128 * (k + 1))
                nc.sync.dma_start(out=c1w[k], in_=d_c1w.ap()[sl, :])
                nc.sync.dma_start(out=c2w[k], in_=d_c2w.ap()[sl, :])
                nc.sync.dma_start(out=fd2w[k], in_=d_fd2w.ap()[sl, :])
                nc.sync.dma_start(out=sd1c[k], in_=d_sd1c.ap()[sl, :])
                nc.sync.dma_start(out=sd2w[k], in_=d_sd2.ap()[sl, :])
                nc.sync.dma_start(out=gw[k], in_=d_gw.ap()[sl, :])
                nc.sync.dma_start(out=outw[k], in_=d_outw.ap()[sl, :])
            sd1p = wtile("sd1p", [SUB, COND])
            nc.sync.dma_start(out=sd1p, in_=d_sd1p.ap())
            sd1pr = wtile("sd1pr", [SUB, COND])
            nc.sync.dma_start(out=sd1pr, in_=d_sd1pr.ap())
            sd1pi = wtile("sd1pi", [SUB, COND])
            nc.sync.dma_start(out=sd1pi, in_=d_sd1pi.ap())
            bcols = wtile("bcols", [128, 12], dt.float32)
            nc.sync.dma_start(out=bcols, in_=d_bcols.ap())
            gib = wtile("gib", [128, 18], dt.float32)
            nc.sync.dma_start(out=gib, in_=d_gib.ap())
            ghb = wtile("ghb", [128, 18], dt.float32)
            nc.sync.dma_start(out=ghb, in_=d_ghb.ap())
            outb = wtile("outb", [SUB, 1], dt.float32)
            nc.sync.dma_start(out=outb, in_=d_outb.ap())

            def bias_col(j):      # (128,1) f32 from packed misc biases
                return bcols[:, j:j + 1]

            # combined r,z bias per GRU: brz_all[:, g*4+m] = gib+ghb (first 4 cols)
            brz = wtile("brz", [128, 12], dt.float32)
            for g in range(3):
                nc.vector.tensor_add(brz[:, 4 * g:4 * g + 4],
                                     gib[:, 6 * g:6 * g + 4], ghb[:, 6 * g:6 * g + 4])

            # ================= Stage A: conditioning =================
            # one-hot of period over 256 rows (2 chunks of 128 partitions)
            pbc = wtile("pbc", [128, BT], dt.float32)
            nc.sync.dma_start(out=pbc, in_=d_prow.ap()[0:1, :].partition_broadcast(128))
            iot2 = wtile("iot2", [128, 2], dt.int32)
            nc.gpsimd.iota(iot2, [[128, 2]], base=0, channel_multiplier=1)
            iot2f = wtile("iot2f", [128, 2], dt.float32)
            nc.vector.tensor_copy(out=iot2f, in_=iot2)
            onehot = [wtile(f"onehot{k}", [128, BT]) for k in range(2)]
            for k in range(2):
                nc.vector.tensor_scalar(out=onehot[k], in0=pbc,
                                        scalar1=iot2f[:, k:k + 1], scalar2=None,
                                        op0=OP.is_equal)
            # peT (64, 832) bf16 = pembed^T @ onehot
            peT = wtile("peT", [PEMB, BT])
            for f in range(2):
                fs = slice(416 * f, 416 * (f + 1))
                ps = psp.tile([PEMB, 416], dt.float32, tag="big", name="big")
                for k in range(2):
                    nc.tensor.matmul(ps, pemb[k], onehot[k][:, fs],
                                     start=(k == 0), stop=(k == 1))
                nc.vector.tensor_copy(out=peT[:, fs], in_=ps)
            # t1T (2 x (128, 832)) = tanh(fd1)
            t1T = [wtile(f"t1T{m}", [128, BT]) for m in range(2)]
            for m in range(2):
                ms = slice(128 * m, 128 * (m + 1))
                for f in range(2):
                    fs = slice(416 * f, 416 * (f + 1))
                    ps = psp.tile([128, 416], dt.float32, tag="big", name="big")
                    nc.tensor.matmul(ps, fd1wA[:, ms], featT[:, fs],
                                     start=True, stop=False)
                    nc.tensor.matmul(ps, fd1wB[:, ms], peT[:, fs],
                                     start=False, stop=True)
                    nc.scalar.activation(t1T[m][:, fs], ps, AF.Tanh, bias=bias_col(m))
            # conv1 -> x1T (2 x (128, 8, 102))
            T1 = T - 2
            x1T = [wtile(f"x1T{m}", [128, BC * T1]) for m in range(2)]
            for m in range(2):
                ms = slice(128 * m, 128 * (m + 1))
                for fb in range(2):
                    ps = psp.tile([128, 4, T1], dt.float32, tag="big", name="big")
                    first = True
                    for d in range(3):
                        for k in range(2):
                            rhs = t1T[k].rearrange("p (b t) -> p b t", b=BC)[
                                :, 4 * fb:4 * fb + 4, d:d + T1]
                            nc.tensor.matmul(
                                ps, c1w[k][:, 256 * d + 128 * m:256 * d + 128 * m + 128],
                                rhs, start=first, stop=(d == 2 and k == 1))
                            first = False
                    nc.scalar.activation(
                        x1T[m][:, 4 * T1 * fb:4 * T1 * (fb + 1)], ps, AF.Tanh,
                        bias=bias_col(2 + m))
            # conv2 -> x2T (2 x (128, 8, 100))
            x2T = [wtile(f"x2T{m}", [128, BC * NBF]) for m in range(2)]
            for m in range(2):
                for fb in range(2):
                    ps = psp.tile([128, 4, NBF], dt.float32, tag="big", name="big")
                    first = True
                    for d in range(3):
                        for k in range(2):
                            rhs = x1T[k].rearrange("p (b t) -> p b t", b=BC)[
                                :, 4 * fb:4 * fb + 4, d:d + NBF]
                            nc.tensor.matmul(
                                ps, c2w[k][:, 256 * d + 128 * m:256 * d + 128 * m + 128],
                                rhs, start=first, stop=(d == 2 and k == 1))
                            first = False
                    nc.scalar.activation(
                        x2T[m][:, 4 * NBF * fb:4 * NBF * (fb + 1)], ps, AF.Tanh,
                        bias=bias_col(4 + m))
            # fd2 -> condT (2 x (128, 800))
            condT = [wtile(f"condT{m}", [128, BC * NBF]) for m in range(2)]
            for m in range(2):
                ms = slice(128 * m, 128 * (m + 1))
                for fb in range(2):
                    fs = slice(400 * fb, 400 * (fb + 1))
                    ps = psp.tile([128, 400], dt.float32, tag="big", name="big")
                    for k in range(2):
                        nc.tensor.matmul(ps, fd2w[k][:, ms], x2T[k][:, fs],
                                         start=(k == 0), stop=(k == 1))
                    nc.scalar.activation(condT[m][:, fs], ps, AF.Tanh,
                                         bias=bias_col(6 + m))

            # ================= Stage B: phase embedding =================
            w0 = wk.tile([BC, NBF], dt.float32, tag="w0", name="w0")
            nc.vector.reciprocal(out=w0, in_=psc[:, 3:3 + NBF])
            w0s = wk.tile([BC, NBF], dt.float32, tag="w0s", name="w0s")   # w0_shift * ... pre-scale
            # w0 currently = 1/p ; w0_true = 2pi/p
            nc.vector.tensor_scalar(out=w0, in0=w0, scalar1=float(TWO_PI),
                                    scalar2=None, op0=OP.mult)
            # w0_shift: col0 = 2pi*rsh/FRAME, cols1: w0[:, :-1]
            nc.vector.tensor_scalar(out=w0s[:, 0:1], in0=rsh,
                                    scalar1=float(TWO_PI / FRAME), scalar2=None,
                                    op0=OP.mult)
            nc.vector.tensor_copy(out=w0s[:, 1:NBF], in_=w0[:, 0:NBF - 1])
            zeros = wk.tile([BC, NBF], dt.float32, tag="zeros", name="zeros")
            nc.vector.memset(zeros, 0.0)
            cum = wk.tile([BC, NBF], dt.float32, tag="cum", name="cum")
            nc.vector.tensor_tensor_scan(out=cum, data0=w0s, data1=zeros,
                                         initial=0.0, op0=OP.add, op1=OP.add)
            cumph = wk.tile([BC, NBF], dt.float32, tag="cumph", name="cumph")
            nc.vector.tensor_scalar(out=cumph, in0=cum, scalar1=float(FRAME),
                                    scalar2=None, op0=OP.mult)
            # reduce mod 2pi: k = int(cumph/2pi); cumr = cumph - 2pi*k
            mm_ = wk.tile([BC, NBF], dt.float32, tag="mm_", name="mm_")
            nc.vector.tensor_scalar(out=mm_, in0=cumph, scalar1=float(1.0 / TWO_PI),
                                    scalar2=None, op0=OP.mult)
            mi = wk.tile([BC, NBF], dt.int32, tag="mi", name="mi")
            nc.vector.tensor_copy(out=mi, in_=mm_)
            mf = wk.tile([BC, NBF], dt.float32, tag="mf", name="mf")
            nc.vector.tensor_copy(out=mf, in_=mi)
            cumr = wk.tile([BC, NBF], dt.float32, tag="cumr", name="cumr")
            nc.vector.scalar_tensor_tensor(out=cumr, in0=mf, scalar=float(-TWO_PI),
                                           in1=cumph, op0=OP.mult, op1=OP.add)
            # per-(b,s) base & w0 (8, 400): s = 4*fi + si
            w0rep = wk.tile([BC, STEPS], dt.float32, tag="w0rep", name="w0rep")
            nc.vector.tensor_copy(
                out=w0rep.rearrange("p (t r) -> p t r", t=NBF),
                in_=w0.unsqueeze(2).broadcast_to((BC, NBF, 4)))
            base8 = wk.tile([BC, STEPS], dt.float32, tag="base8", name="base8")
            si40 = wk.tile([BC, STEPS], dt.float32, tag="si40", name="si40")
            nc.gpsimd.iota(si40.rearrange("p (t r) -> p t r", t=NBF),
                           [[0, NBF], [SUB, 4]], base=0, channel_multiplier=0,
                           allow_small_or_imprecise_dtypes=True)
            nc.vector.tensor_mul(si40, si40, w0rep)      # si*40*w0
            nc.vector.tensor_tensor(
                out=base8.rearrange("p (t r) -> p t r", t=NBF),
                in0=si40.rearrange("p (t r) -> p t r", t=NBF),
                in1=cumr.unsqueeze(2).broadcast_to((BC, NBF, 4)), op=OP.add)
            # flatten to rows and broadcast to 40 partitions
            d_scr = nc.dram_tensor("scr", [2, BC * STEPS], dt.float32)
            for b in range(BC):
                bs = slice(STEPS * b, STEPS * (b + 1))
                nc.sync.dma_start(out=d_scr.ap()[0:1, bs], in_=w0rep[b:b + 1, :])
                nc.sync.dma_start(out=d_scr.ap()[1:2, bs], in_=base8[b:b + 1, :])
            w0bc = wk.tile([SUB, BC * STEPS], dt.float32, tag="w0bc", name="w0bc")
            basebc = wk.tile([SUB, BC * STEPS], dt.float32, tag="basebc", name="basebc")
            nc.sync.dma_start(out=w0bc, in_=d_scr.ap()[0:1, :].partition_broadcast(SUB))
            nc.sync.dma_start(out=basebc, in_=d_scr.ap()[1:2, :].partition_broadcast(SUB))
            jv = wk.tile([SUB, 1], dt.int32, tag="jv", name="jv")
            nc.gpsimd.iota(jv, [[0, 1]], base=0, channel_multiplier=1)
            jvf = wk.tile([SUB, 1], dt.float32, tag="jvf", name="jvf")
            nc.vector.tensor_copy(out=jvf, in_=jv)
            ang = wk.tile([SUB, BC * STEPS], dt.float32, tag="ang", name="ang")
            nc.vector.scalar_tensor_tensor(out=ang, in0=w0bc, scalar=jvf,
                                           in1=basebc, op0=OP.mult, op1=OP.add)
            # range-reduce both angles into [-pi, pi] for the ACT Sin table:
            #   r = round(x/2pi) via (x*inv2pi + 2^23) - 2^23 ; x' = x - 2pi*r
            MAGIC = float(1.5 * 2.0 ** 23)
            prT = wtile("prT", [SUB, BC * STEPS])
            piT = wtile("piT", [SUB, BC * STEPS])
            angc = wk.tile([SUB, BC * STEPS], dt.float32, tag="angc", name="angc")
            for is_cos in (True, False):
                if is_cos:
                    nc.vector.tensor_scalar(out=angc, in0=ang,
                                            scalar1=float(np.pi / 2), scalar2=None,
                                            op0=OP.add)
                    src = angc
                else:
                    src = ang
                sfx = "c" if is_cos else "s"
                rr = wk.tile([SUB, BC * STEPS], dt.float32, tag=f"rr{sfx}",
                             name=f"rr{sfx}")
                nc.vector.tensor_scalar(out=rr, in0=src,
                                        scalar1=float(1.0 / TWO_PI), scalar2=MAGIC,
                                        op0=OP.mult, op1=OP.add)
                nc.vector.tensor_scalar(out=rr, in0=rr, scalar1=MAGIC,
                                        scalar2=None, op0=OP.subtract)
                dst = prT if is_cos else piT
                red = wk.tile([SUB, BC * STEPS], dt.float32, tag=f"red{sfx}",
                              name=f"red{sfx}")
                nc.vector.scalar_tensor_tensor(out=red, in0=rr,
                                               scalar=float(-TWO_PI), in1=src,
                                               op0=OP.mult, op1=OP.add)
                nc.scalar.activation(dst, red, AF.Sin)
                if _dbg:
                    nc.sync.dma_start(out=d_dbg[f"dbg_red{sfx}"].ap(), in_=red)
                    nc.sync.dma_start(out=d_dbg[f"dbg_rr{sfx}"].ap(), in_=rr)

            # ================= Stage C: A[s] precompute =================
            # AT (128, 2, 8, nsteps) fp32 : free = c*(8*nsteps) + b*nsteps + s
            AT = wp.tile([128, 2 * BC * nsteps], dt.float32, tag="AT", name="AT")
            ATv = AT.rearrange("p (c b s) -> p c b s", c=2, b=BC)
            nfr = (nsteps + 3) // 4
            assert nsteps % 4 == 0
            for c in range(2):
                cs = slice(128 * c, 128 * (c + 1))
                for b in range(BC):
                    ps = psp.tile([128, nfr, 4], dt.float32, tag="big", name="big")
                    for k in range(2):
                        rhs = condT[k].rearrange("p (b t) -> p b t", b=BC)[
                            :, b, 0:nfr].unsqueeze(2).broadcast_to((128, nfr, 4))
                        nc.tensor.matmul(ps, sd1c[k][:, cs], rhs,
                                         start=(k == 0), stop=False)
                    psf = ps.rearrange("p t r -> p (t r)")
                    nc.tensor.matmul(psf, sd1pr[:, cs],
                                     prT[:, b * STEPS:b * STEPS + nsteps],
                                     start=False, stop=False)
                    nc.tensor.matmul(psf, sd1pi[:, cs],
                                     piT[:, b * STEPS:b * STEPS + nsteps],
                                     start=False, stop=True)
                    nc.vector.tensor_scalar(
                        out=ATv[:, c, b, :], in0=psf,
                        scalar1=bias_col(8 + c), scalar2=None, op0=OP.add)

            # ================= Stage D: recurrence =================
            prevb = wp.tile([SUB, BC], dt.bfloat16, tag="prevb", name="prevb")
            nc.vector.memset(prevb, 0.0)
            hF = [wp.tile([128, 16], dt.float32, tag=f"hF{g}", name=f"hF{g}") for g in range(3)]
            hB = [wp.tile([128, 16], dt.bfloat16, tag=f"hB{g}", name=f"hB{g}") for g in range(3)]
            for g in range(3):
                nc.vector.memset(hF[g], 0.0)
                nc.vector.memset(hB[g], 0.0)
            outT = wp.tile([SUB, BC * nsteps], dt.float32, tag="outT", name="outT")
            outTv = outT.rearrange("p (b s) -> p b s", b=BC)

            with tc.tile_pool(name="lps", bufs=1, space="PSUM") as lp, \
                 tc.tile_pool(name="lw", bufs=3) as lwk:

                def gw_lhs(g, s, m):  # (128,128) weight tile view, k-chunk in list idx
                    col = (g * 2 + s) * 768 + 128 * m
                    return [gw[k][:, col:col + 128] for k in range(2)]

                for s in range(nsteps):
                    # ---- sd1: prev part + A[s], tanh ----
                    tt_ps = lp.tile([128, 2, BC], dt.float32, tag="tt", name="tt")
                    for c in range(2):
                        nc.tensor.matmul(tt_ps[:, c, :],
                                         sd1p[:, 128 * c:128 * (c + 1)], prevb,
                                         start=True, stop=True)
                    ttpre = lwk.tile([128, 2, BC], dt.float32, tag="ttpre", name="ttpre")
                    nc.vector.tensor_tensor(out=ttpre, in0=tt_ps,
                                            in1=ATv[:, :, :, s], op=OP.add)
                    ttT = lwk.tile([128, 2, BC], dt.bfloat16, tag="ttT", name="ttT")
                    nc.scalar.activation(ttT, ttpre, AF.Tanh)
                    # ---- sd2 ----
                    tt2_ps = lp.tile([128, 2, BC], dt.float32, tag="tt2", name="tt2")
                    for c in range(2):
                        for k in range(2):
                            nc.tensor.matmul(
                                tt2_ps[:, c, :], sd2w[k][:, 128 * c:128 * (c + 1)],
                                ttT[:, k, :], start=(k == 0), stop=(k == 1))
                    xT = lwk.tile([128, 2, BC], dt.bfloat16, tag="xT", name="xT")
                    for c in range(2):
                        nc.scalar.activation(xT[:, c, :], tt2_ps[:, c, :], AF.Tanh,
                                             bias=bias_col(10 + c))
                    # ---- GRUs ----
                    hBv = [hB[g].rearrange("p (c b) -> p c b", c=2) for g in range(3)]
                    for g in range(3):
                        xin = xT if g == 0 else hBv[g - 1]
                        rz_ps = lp.tile([128, 4, BC], dt.float32, tag="rz", name="rz")
                        for m in range(4):
                            wi = gw_lhs(g, 0, m)
                            wh = gw_lhs(g, 1, m)
                            for k in range(2):
                                nc.tensor.matmul(rz_ps[:, m, :], wi[k], xin[:, k, :],
                                                 start=(k == 0), stop=False)
                            for k in range(2):
                                nc.tensor.matmul(rz_ps[:, m, :], wh[k], hBv[g][:, k, :],
                                                 start=False, stop=(k == 1))
                        gin_ps = lp.tile([128, 2, BC], dt.float32, tag="gin", name="gin")
                        ghn_ps = lp.tile([128, 2, BC], dt.float32, tag="ghn", name="ghn")
                        for m in range(2):
                            wi = gw_lhs(g, 0, 4 + m)
                            wh = gw_lhs(g, 1, 4 + m)
                            for k in range(2):
                                nc.tensor.matmul(gin_ps[:, m, :], wi[k], xin[:, k, :],
                                                 start=(k == 0), stop=(k == 1))
                            for k in range(2):
                                nc.tensor.matmul(ghn_ps[:, m, :], wh[k], hBv[g][:, k, :],
                                                 start=(k == 0), stop=(k == 1))
                        rzT = lwk.tile([128, 4, BC], dt.float32, tag="rzT", name="rzT")
                        for m in range(4):
                            nc.scalar.activation(rzT[:, m, :], rz_ps[:, m, :],
                                                 AF.Sigmoid,
                                                 bias=brz[:, 4 * g + m:4 * g + m + 1])
                        narg = lwk.tile([128, 2, BC], dt.float32, tag="narg", name="narg")
                        t1 = lwk.tile([128, 2, BC], dt.float32, tag="t1", name="t1")
                        for c in range(2):
                            nc.vector.scalar_tensor_tensor(
                                out=t1[:, c, :], in0=ghn_ps[:, c, :],
                                scalar=ghb[:, 6 * g + 4 + c:6 * g + 5 + c],
                                in1=rzT[:, c, :], op0=OP.add, op1=OP.mult)
                            nc.vector.scalar_tensor_tensor(
                                out=narg[:, c, :], in0=gin_ps[:, c, :],
                                scalar=gib[:, 6 * g + 4 + c:6 * g + 5 + c],
                                in1=t1[:, c, :], op0=OP.add, op1=OP.add)
                        nT = lwk.tile([128, 2, BC], dt.float32, tag="nT", name="nT")
                        nc.scalar.activation(nT, narg, AF.Tanh)
                        hFv = hF[g].rearrange("p (c b) -> p c b", c=2)
                        d_ = lwk.tile([128, 2, BC], dt.float32, tag="d_", name="d_")
                        nc.vector.tensor_sub(d_, hFv, nT)
                        zd = lwk.tile([128, 2, BC], dt.float32, tag="zd", name="zd")
                        nc.vector.tensor_mul(zd, rzT[:, 2:4, :], d_)
                        nc.vector.tensor_tensor(out=hFv, in0=nT, in1=zd, op=OP.add)
                        nc.vector.tensor_copy(out=hBv[g], in_=hFv)
                    # ---- out ----
                    o_ps = lp.tile([SUB, BC], dt.float32, tag="o", name="o")
                    for k in range(2):
                        nc.tensor.matmul(o_ps, outw[k], hBv[2][:, k, :],
                                         start=(k == 0), stop=(k == 1))
                    nc.scalar.activation(outTv[:, :, s], o_ps, AF.Tanh, bias=outb)
                    nc.vector.tensor_copy(out=prevb, in_=outTv[:, :, s])

            if _dbg:
                nc.sync.dma_start(out=d_dbg["dbg_peT"].ap(), in_=peT)
                nc.sync.dma_start(out=d_dbg["dbg_condT0"].ap(), in_=condT[0])
                nc.sync.dma_start(out=d_dbg["dbg_prT"].ap(), in_=prT)
                nc.sync.dma_start(out=d_dbg["dbg_piT"].ap(), in_=piT)
                nc.sync.dma_start(out=d_dbg["dbg_base8"].ap(), in_=base8)
                nc.sync.dma_start(out=d_dbg["dbg_w0rep"].ap(), in_=w0rep)
                nc.sync.dma_start(out=d_dbg["dbg_AT"].ap(), in_=AT)
                nc.sync.dma_start(out=d_dbg["dbg_onehot0"].ap(), in_=onehot[0])
            # ---------------- epilogue ----------------
            nc.sync.dma_start(out=d_sig.ap(), in_=outT)
            for g in range(3):
                nc.sync.dma_start(out=d_h[g].ap(), in_=hF[g])
    nc.compile()
    return nc


def _prep_inputs(features, period, rand_shift, pembed, fd1_w, conv1_w, conv2_w,
                 fd2_w, sd1_w, sd2_w, gws, out_w, biases):
    """Build the 8 per-core input maps (layout/dtype prep only)."""
    f32 = np.float32
    shared = {}
    shared["pembed"] = np.ascontiguousarray(pembed.astype(BF16))
    shared["fd1w"] = np.ascontiguousarray(fd1_w.astype(BF16))
    shared["c1w"] = np.ascontiguousarray(
        conv1_w.transpose(1, 2, 0).reshape(COND, 3 * COND).astype(BF16))
    shared["c2w"] = np.ascontiguousarray(
        conv2_w.transpose(1, 2, 0).reshape(COND, 3 * COND).astype(BF16))
    shared["fd2w"] = np.ascontiguousarray(fd2_w.astype(BF16))
    shared["sd1c"] = np.ascontiguousarray(sd1_w[0:COND].astype(BF16))
    shared["sd1p"] = np.ascontiguousarray(sd1_w[COND:COND + SUB].astype(BF16))
    shared["sd1pr"] = np.ascontiguousarray(
        sd1_w[COND + SUB:COND + 2 * SUB].astype(BF16))
    shared["sd1pi"] = np.ascontiguousarray(sd1_w[COND + 2 * SUB:].astype(BF16))
    shared["sd2"] = np.ascontiguousarray(sd2_w.astype(BF16))
    gw = np.concatenate(gws, axis=1)  # (256, 6*768) in order g1i,g1h,g2i,g2h,g3i,g3h
    shared["gw"] = np.ascontiguousarray(gw.astype(BF16))
    shared["outw"] = np.ascontiguousarray(out_w.astype(BF16))
    (fd1_b, c1b, c2b, fd2_b, sd1_b, sd2_b, gibs, ghbs, out_b) = biases
    bcols = np.stack([x.reshape(2, 128).T for x in
                      (fd1_b, c1b, c2b, fd2_b, sd1_b, sd2_b)], axis=1)
    shared["bcols"] = np.ascontiguousarray(
        bcols.reshape(128, 12).astype(f32))
    shared["gib"] = np.ascontiguousarray(np.concatenate(
        [x.reshape(6, 128).T for x in gibs], axis=1).astype(f32))
    shared["ghb"] = np.ascontiguousarray(np.concatenate(
        [x.reshape(6, 128).T for x in ghbs], axis=1).astype(f32))
    shared["outb"] = np.ascontiguousarray(out_b.reshape(SUB, 1).astype(f32))

    in_maps = []
    for c in range(NCORES):
        sl = slice(BC * c, BC * (c + 1))
        m = dict(shared)
        m["featT"] = np.ascontiguousarray(
            features[sl].transpose(2, 0, 1).reshape(FEAT, BT).astype(BF16))
        pf = period[sl].astype(f32)
        m["prow"] = np.ascontiguousarray(pf.reshape(1, BT))
        m["psc"] = np.ascontiguousarray(pf)
        m["rsh"] = np.ascontiguousarray(rand_shift[sl].astype(f32))
        in_maps.append(m)
    return in_maps


def kernel(features, period, nb_frames, rand_shift, pembed, fd1_w, fd1_b,
           conv1_w, conv1_b, conv2_w, conv2_b, fd2_w, fd2_b, sd1_w, sd1_b,
           sd2_w, sd2_b, g1_wi, g1_bi, g1_wh, g1_bh, g2_wi, g2_bi, g2_wh,
           g2_bh, g3_wi, g3_bi, g3_wh, g3_bh, out_w, out_b, _nsteps=STEPS,
           _trace=False):
    from concourse.bass_utils import run_bass_kernel_spmd

    features = np.asarray(features)
    period = np.asarray(period)
    assert int(nb_frames) == NBF and features.shape == (B, T, FEAT)

    key = _nsteps
    if key not in _cache:
        _cache[key] = _build(_nsteps)
    nc = _cache[key]

    in_maps = _prep_inputs(
        features, period, np.asarray(rand_shift), np.asarray(pembed),
        np.asarray(fd1_w), np.asarray(conv1_w), np.asarray(conv2_w),
        np.asarray(fd2_w), np.asarray(sd1_w), np.asarray(sd2_w),
        [np.asarray(x) for x in (g1_wi, g1_wh, g2_wi, g2_wh, g3_wi, g3_wh)],
        np.asarray(out_w),
        ([np.asarray(x) for x in (fd1_b, conv1_b, conv2_b, fd2_b, sd1_b, sd2_b)] +
         [[np.asarray(g1_bi), np.asarray(g2_bi), np.asarray(g3_bi)],
          [np.asarray(g1_bh), np.asarray(g2_bh), np.asarray(g3_bh)],
          np.asarray(out_b)]))

    res = run_bass_kernel_spmd(nc, in_maps, core_ids=list(range(NCORES)),
                               trace=_trace)
    sig = np.empty((B, _nsteps * SUB), np.float32)
    hs = [np.empty((B, COND), np.float32) for _ in range(3)]
    for c in range(NCORES):
        r = res.results[c]
        o = r["sig"].reshape(SUB, BC, _nsteps)
        sig[BC * c:BC * (c + 1)] = o.transpose(1, 2, 0).reshape(BC, _nsteps * SUB)
        for g in range(3):
            hs[g][BC * c:BC * (c + 1)] = (
                r[f"h{g}"].reshape(128, 2, BC).transpose(2, 1, 0).reshape(BC, COND))
    kernel._last_result = res
    return sig, (hs[0], hs[1], hs[2])
